# revision 28
# baseline (speedup 1.0000x reference)
"""TRN2 Bass kernel for nn_Attention_23493471109551 (v2, fp16).

Full attention layer: QKV projections + interleaved RoPE + causal softmax
attention + output projection, for B=4, S=2048, D=1024, H=16, Dh=64, fp32 I/O.

Sharding: 8 cores = 4 batches x 2 head-groups (8 heads each).  Each core
computes its batch/head-group's attention and a partial output projection
(W_o row-block); host sums the two partials per batch.

v2 changes vs v1 (fp32r baseline, 707us):
  - all matmul operands fp16 (err 3.4e-4 vs 2e-2 budget): halves HBM traffic,
    removes the fp32r n<256 4x penalty, faster ldweights.
  - weights hoisted to SBUF once (v1 reloaded W_q/W_k 4x: 25MB extra DMA).
  - causal mask applied as a 0/1 fp16 multiply on the exp output (SBUF)
    instead of -1e30 add on the score PSUM: cheaper and shortens the
    PSUM critical path.
  - softmax denominator reciprocal via reciprocal_approx_fast (v1's
    nc.vector.reciprocal was 3.3us per call, 106us total DVE).
  - per-chunk emission interleave: projection chunk c+1 is emitted between
    attention chunk c and its output projection, so the tile scheduler can
    fill the (Activation-bound) attention phase with projection matmuls and
    keep the PE continuously busy at its top p-state.

Layout (per core):
  qpT/kpT: [dh-on-partitions (2 heads x 64), hp, S] fp16
  scoresT [sk, sq] in PSUM; exp'd (scale fused) to fp16 et; PV feeds from et
  directly; denominator = ones-column appended to V (row 64 of the PV psum);
  normalization = approx-reciprocal + gpsimd partition_broadcast + fp16 mul.
"""
import math
import numpy as np

import concourse.bass as bass
import concourse.tile as tile
import concourse.mybir as mybir
from concourse import bacc, bass_utils

# problem constants
B, S, D = 4, 2048, 1024
H, Dh = 16, 64
EQ, EV = 2048, 1024          # q/k and v input feature dims
F = 512                      # features per core (8 heads x 64)
P = 128
N_CORES = 8
SCALE = 1.0 / math.sqrt(D)   # 1/32
ROPE_BASE = 10000.0
SWAP_MASK = [i ^ 1 for i in range(32)]

F16 = mybir.dt.float16
F32 = mybir.dt.float32

# test hooks (harness ignores these)
KERNEL_TRACE = False
LAST_RESULT = None

_nc_cache = None


def _build_nc():
    nc = bacc.Bacc("TRN2", target_bir_lowering=False, debug=False)
    qT = nc.dram_tensor("qT", [EQ, S], F16, kind="ExternalInput").ap()
    kT = nc.dram_tensor("kT", [EQ, S], F16, kind="ExternalInput").ap()
    vT = nc.dram_tensor("vT", [EV, S], F16, kind="ExternalInput").ap()
    wqT = nc.dram_tensor("wqT", [EQ, F], F16, kind="ExternalInput").ap()
    wkT = nc.dram_tensor("wkT", [EQ, F], F16, kind="ExternalInput").ap()
    wvT = nc.dram_tensor("wvT", [EV, F], F16, kind="ExternalInput").ap()
    woT = nc.dram_tensor("woT", [F, D], F16, kind="ExternalInput").ap()
    cosf = nc.dram_tensor("cosf", [P, S], F16, kind="ExternalInput").ap()
    sinf = nc.dram_tensor("sinf", [P, S], F16, kind="ExternalInput").ap()
    maskA = nc.dram_tensor("maskA", [P, P], F16, kind="ExternalInput").ap()
    out = nc.dram_tensor("out", [S, D], F32, kind="ExternalOutput").ap()

    EXP = mybir.ActivationFunctionType.Exp

    with tile.TileContext(nc) as tc:
        with (
            tc.tile_pool(name="consts", bufs=1) as consts,
            tc.tile_pool(name="persist", bufs=1) as persist,
            tc.tile_pool(name="insb", bufs=2) as insb_pool,
            tc.tile_pool(name="vsb", bufs=2) as vsb_pool,
            tc.tile_pool(name="rope", bufs=2) as rope_pool,
            tc.tile_pool(name="et", bufs=3) as et_pool,
            tc.tile_pool(name="norm", bufs=2) as norm_pool,
            tc.tile_pool(name="denp", bufs=1) as den_pool,
            tc.tile_pool(name="attnc", bufs=2) as attnc_pool,
            tc.tile_pool(name="outsb", bufs=2) as out_pool,
            tc.tile_pool(name="projps", bufs=2, space="PSUM") as proj_ps,
            tc.tile_pool(name="scps", bufs=2, space="PSUM") as sc_ps,
            tc.tile_pool(name="pops", bufs=2, space="PSUM") as po_ps,
        ):
            # ---- persistent activations
            qpT = persist.tile([P, 4, S], F16, tag="qpT")
            kpT = persist.tile([P, 4, S], F16, tag="kpT")
            vpa = persist.tile([P, 16, 8, 65], F16, tag="vpa")
            nc.vector.memset(vpa[:, :, :, 64:65], 1.0)  # softmax-denominator ones

            # ---- weights + tables, loaded once
            wq_t = consts.tile([P, 16, F], F16, tag="wq")
            wk_t = consts.tile([P, 16, F], F16, tag="wk")
            wv_t = consts.tile([P, 8, F], F16, tag="wv")
            wo_t = consts.tile([P, 4, D], F16, tag="wo")
            cos_t = consts.tile([P, S], F16, tag="cos")
            sin_t = consts.tile([P, S], F16, tag="sin")
            mask_t = consts.tile([P, P], F16, tag="mask")
            # 3D views of the HBM inputs: [p, e, s]
            qTr = qT.rearrange("(e p) s -> p e s", p=P)
            kTr = kT.rearrange("(e p) s -> p e s", p=P)
            vTr = vT.rearrange("(e p) s -> p e s", p=P)
            wqTr = wqT.rearrange("(e p) f -> p e f", p=P)
            wkTr = wkT.rearrange("(e p) f -> p e f", p=P)
            wvTr = wvT.rearrange("(e p) f -> p e f", p=P)
            woTr = woT.rearrange("(e p) f -> p e f", p=P)

            def bulk(dmae, dst, src, n_e, split=4):
                step = n_e // split
                for i in range(0, n_e, step):
                    dmae.dma_start(dst[:, i:i + step], src[:, i:i + step])

            def load_qk(c, split=4):
                ssl = slice(c * 512, (c + 1) * 512)
                qsb = insb_pool.tile([P, 16, 512], F16, tag="insb", name=f"qsb{c}")
                bulk(nc.sync, qsb, qTr[:, :, ssl], 16, split)
                ksb = insb_pool.tile([P, 16, 512], F16, tag="insb", name=f"ksb{c}")
                bulk(nc.scalar, ksb, kTr[:, :, ssl], 16, split)
                return qsb, ksb

            def load_v(stq):
                vsb = vsb_pool.tile([P, 8, 512], F16, tag="vsb", name=f"vsb{stq}")
                bulk(nc.gpsimd, vsb, vTr[:, :, stq * 512:(stq + 1) * 512], 8, 2)
                return vsb

            # startup loads, ordered by chunk-0 criticality per queue:
            # gpsimd: wq -> wv -> vsb0 ; sync: qsb0 -> wo, mask ;
            # scalar: cos, sin -> ksb0 -> wk
            bulk(nc.gpsimd, wq_t, wqTr, 16, 8)
            nc.scalar.dma_start(cos_t[:], cosf)
            nc.scalar.dma_start(sin_t[:], sinf)
            qsb0, ksb0 = load_qk(0, split=8)
            bulk(nc.scalar, wk_t, wkTr, 16)
            bulk(nc.gpsimd, wv_t, wvTr, 8, 2)
            vsb0 = load_v(0)
            bulk(nc.sync, wo_t, woTr, 4, 2)
            nc.sync.dma_start(mask_t[:], maskA)

            def proj_qk(c, qsb, ksb):
                """Project q and k for s-chunk c (fp16 matmuls) + rope."""
                ssl = slice(c * 512, (c + 1) * 512)
                for xsb, w_t, dstT, nm in ((qsb, wq_t, qpT, "q"),
                                           (ksb, wk_t, kpT, "k")):
                    for ci in range(4):
                        ps = proj_ps.tile([P, 512], F32, tag="proj",
                                          name=f"ps{nm}{c}_{ci}")
                        for e in range(16):
                            nc.tensor.matmul(ps[:], w_t[:, e, ci * P:(ci + 1) * P],
                                             xsb[:, e, :],
                                             start=(e == 0), stop=(e == 15))
                        # rope: out = x*cos + pairswap(x*sin')
                        a16 = rope_pool.tile([P, 512], F16, tag="ropeA")
                        nc.scalar.copy(a16[:], ps[:])
                        cm = rope_pool.tile([P, 512], F16, tag="ropeC")
                        nc.vector.tensor_mul(cm[:], a16[:], cos_t[:, ssl])
                        sm = rope_pool.tile([P, 512], F16, tag="ropeS")
                        nc.vector.tensor_mul(sm[:], a16[:], sin_t[:, ssl])
                        sm2 = rope_pool.tile([P, 512], F16, tag="ropeS2")
                        nc.vector.stream_shuffle(sm2[:], sm[:], SWAP_MASK)
                        nc.vector.tensor_add(dstT[:, ci, ssl], cm[:], sm2[:])

            def proj_v(stq, vsb):
                """Project v for s-chunk stq into vpa (s on partitions)."""
                for j in range(4):
                    ps = proj_ps.tile([P, 512], F32, tag="proj",
                                      name=f"psv{stq}_{j}")
                    for e in range(8):
                        nc.tensor.matmul(ps[:], vsb[:, e, j * P:(j + 1) * P],
                                         wv_t[:, e, :],
                                         start=(e == 0), stop=(e == 7))
                    st = stq * 4 + j
                    nc.scalar.copy(vpa[:, st, :, 0:64],
                                   ps[:].rearrange("p (h d) -> p h d", h=8))

            def attn_hp(c, hp, attn_c):
                """Causal attention for query chunk c, head-pair hp."""
                nt = 4 * (c + 1)
                if True:
                    po_a = po_ps.tile([P, 512], F32, tag="po", name=f"poa{c}_{hp}")
                    po_b = po_ps.tile([P, 512], F32, tag="po", name=f"pob{c}_{hp}")
                    for t in range(nt):
                        tsl = slice(t * P, (t + 1) * P)
                        rr = P * (t - 4 * c) if t >= 4 * c else 0
                        qsl = slice(c * 512 + rr, (c + 1) * 512)
                        ps_s = sc_ps.tile([P, 2, 512], F32, tag="sc",
                                          name=f"scs{c}_{hp}_{t}")
                        nc.tensor.matmul(ps_s[:, 0, rr:512], kpT[0:64, hp, tsl],
                                         qpT[0:64, hp, qsl], start=True, stop=True)
                        nc.tensor.matmul(ps_s[:, 1, rr:512], kpT[64:128, hp, tsl],
                                         qpT[64:128, hp, qsl], start=True, stop=True)
                        et = et_pool.tile([P, 2, 512], F16, tag="et")
                        nc.scalar.activation(et[:, :, rr:512], ps_s[:, :, rr:512],
                                             EXP, scale=SCALE)
                        if t >= 4 * c:
                            # zero the above-diagonal triangle of this block
                            nc.vector.tensor_mul(
                                et[:, :, rr:rr + P], et[:, :, rr:rr + P],
                                mask_t[:, None, :].to_broadcast((P, 2, P)))
                        nc.tensor.matmul(po_a[0:65, rr:512], vpa[:, t, 2 * hp, :],
                                         et[:, 0, rr:512],
                                         start=(t == 0), stop=(t == nt - 1))
                        nc.tensor.matmul(po_b[0:65, rr:512], vpa[:, t, 2 * hp + 1, :],
                                         et[:, 1, rr:512],
                                         start=(t == 0), stop=(t == nt - 1))
                    # normalize: attn = po[0:64] * (1 / po[64]) per head.
                    # All DVE/gpsimd ops keep inputs at base partition 0
                    # (cross-base inputs miscompile; out-offset is fine).
                    denA = den_pool.tile([1, 512], F32, tag="denA")
                    nc.vector.tensor_copy(denA[:], po_a[64:65, :])
                    denB = den_pool.tile([1, 512], F32, tag="denB")
                    nc.vector.tensor_copy(denB[:], po_b[64:65, :])
                    rcfa = den_pool.tile([1, 512], F32, tag="rcfa")
                    nc.vector.reciprocal_approx_fast(out=rcfa[:], in_=denA[:])
                    rcfb = den_pool.tile([1, 512], F32, tag="rcfb")
                    nc.vector.reciprocal_approx_fast(out=rcfb[:], in_=denB[:])
                    # stage po down to fp16 SBUF immediately so the PSUM bank
                    # frees before the (longer) reciprocal/broadcast chain
                    posbA = norm_pool.tile([64, 512], F16, tag="posbA")
                    nc.vector.tensor_copy(posbA[:], po_a[0:64, :])
                    posbB = norm_pool.tile([64, 512], F16, tag="posbB")
                    nc.vector.tensor_copy(posbB[:], po_b[0:64, :])
                    bcA = norm_pool.tile([64, 512], F32, tag="bcA")
                    nc.gpsimd.partition_broadcast(bcA[:], rcfa[:])
                    bcB = norm_pool.tile([64, 512], F32, tag="bcB")
                    nc.gpsimd.partition_broadcast(bcB[:], rcfb[:])
                    nc.vector.tensor_mul(attn_c[0:64, hp, :], posbA[:], bcA[:])
                    nc.vector.tensor_mul(attn_c[64:128, hp, :], posbB[:], bcB[:])

            def outproj(c, attn_c):
                for j in range(4):
                    pw = [po_ps.tile([P, 512], F32, tag="po", name=f"pw{c}_{j}_{i}")
                          for i in range(2)]
                    for ci in range(4):
                        for oc in range(2):
                            nc.tensor.matmul(pw[oc][:],
                                             attn_c[:, ci, j * P:(j + 1) * P],
                                             wo_t[:, ci, oc * 512:(oc + 1) * 512],
                                             start=(ci == 0), stop=(ci == 3))
                    row = (4 * c + j) * P
                    for oc in range(2):
                        ot = out_pool.tile([P, 512], F32, tag="ot")
                        nc.vector.tensor_copy(ot[:], pw[oc][:])
                        nc.sync.dma_start(out[row:row + P, oc * 512:(oc + 1) * 512],
                                          ot[:])

            # ---- program: chunk 0 projections; per chunk: attention with
            # next-chunk projections emitted alongside (scheduler filler for
            # the PE); chunks 2 and 3 attention interleaved at head-pair
            # granularity so the combined window stays PE-bound (chunk 3
            # alone is exp/Activation-bound).
            proj_qk(0, qsb0, ksb0)
            proj_v(0, vsb0)
            attn_cs = []
            for c in range(2):
                attn_c = attnc_pool.tile([P, 4, 512], F16, tag="attn",
                                         name=f"attn{c}")
                attn_cs.append(attn_c)
                for hp in range(4):
                    attn_hp(c, hp, attn_c)
                qsb, ksb = load_qk(c + 1)
                vsb = load_v(c + 1)
                proj_qk(c + 1, qsb, ksb)
                proj_v(c + 1, vsb)
                outproj(c, attn_c)
            attn_c2 = attnc_pool.tile([P, 4, 512], F16, tag="attn", name="attn2")
            attn_c3 = attnc_pool.tile([P, 4, 512], F16, tag="attn", name="attn3")
            attn_hp(2, 0, attn_c2)
            qsb, ksb = load_qk(3)
            vsb = load_v(3)
            proj_qk(3, qsb, ksb)
            proj_v(3, vsb)
            attn_hp(2, 1, attn_c2)
            attn_hp(3, 0, attn_c3)
            attn_hp(2, 2, attn_c2)
            attn_hp(3, 1, attn_c3)
            attn_hp(2, 3, attn_c2)
            attn_hp(3, 2, attn_c3)
            outproj(2, attn_c2)
            attn_hp(3, 3, attn_c3)
            outproj(3, attn_c3)
    nc.compile()
    return nc


def _tables():
    inv = (1.0 / (ROPE_BASE ** (np.arange(0, Dh, 2, dtype=np.float32) / Dh))
           ).astype(np.float32)                      # [32]
    pos = np.arange(S, dtype=np.float32)
    ang = pos[:, None] * inv[None, :]                # [S, 32]
    cos = np.cos(ang).astype(np.float32)
    sin = np.sin(ang).astype(np.float32)
    d = np.arange(P) % Dh
    i = d // 2
    cosf = np.ascontiguousarray(cos[:, i].T).astype(np.float16)   # [128, S]
    sgn = np.where(d % 2 == 0, 1.0, -1.0).astype(np.float32)
    sinf = np.ascontiguousarray(sin[:, i].T * sgn[:, None]).astype(np.float16)

    p = np.arange(P)
    j = np.arange(P)
    maskA = np.where(p[:, None] <= j[None, :], 1.0, 0.0).astype(np.float16)
    return cosf, sinf, maskA


def kernel(q, k, v, W_q, W_k, W_v, W_o):
    global _nc_cache, LAST_RESULT
    if _nc_cache is None:
        _nc_cache = _build_nc()
    nc = _nc_cache

    cosf, sinf, maskA = _tables()
    q = np.asarray(q, dtype=np.float32)
    k = np.asarray(k, dtype=np.float32)
    v = np.asarray(v, dtype=np.float32)
    W_q = np.asarray(W_q, dtype=np.float32)
    W_k = np.asarray(W_k, dtype=np.float32)
    W_v = np.asarray(W_v, dtype=np.float32)
    W_o = np.asarray(W_o, dtype=np.float32)

    in_maps = []
    for b in range(B):
        qTb = np.ascontiguousarray(q[b].T).astype(np.float16)
        kTb = np.ascontiguousarray(k[b].T).astype(np.float16)
        vTb = np.ascontiguousarray(v[b].T).astype(np.float16)
        for g in range(2):
            fs = slice(g * F, (g + 1) * F)
            in_maps.append({
                "qT": qTb, "kT": kTb, "vT": vTb,
                "wqT": np.ascontiguousarray(W_q[fs, :].T).astype(np.float16),
                "wkT": np.ascontiguousarray(W_k[fs, :].T).astype(np.float16),
                "wvT": np.ascontiguousarray(W_v[fs, :].T).astype(np.float16),
                "woT": np.ascontiguousarray(W_o[:, fs].T).astype(np.float16),
                "cosf": cosf, "sinf": sinf, "maskA": maskA,
            })

    res = bass_utils.run_bass_kernel_spmd(
        nc, in_maps, core_ids=list(range(N_CORES)), trace=KERNEL_TRACE)
    LAST_RESULT = res

    final = np.empty((B, S, D), dtype=np.float32)
    for b in range(B):
        final[b] = res.results[2 * b]["out"] + res.results[2 * b + 1]["out"]
    return final


# revision 29
# speedup vs baseline: 1.0390x; 1.0390x over previous
"""TRN2 Bass kernel for nn_Attention_23493471109551 (v2, fp16).

Full attention layer: QKV projections + interleaved RoPE + causal softmax
attention + output projection, for B=4, S=2048, D=1024, H=16, Dh=64, fp32 I/O.

Sharding: 8 cores = 4 batches x 2 head-groups (8 heads each).  Each core
computes its batch/head-group's attention and a partial output projection
(W_o row-block); host sums the two partials per batch.

v2 changes vs v1 (fp32r baseline, 707us):
  - all matmul operands fp16 (err 3.4e-4 vs 2e-2 budget): halves HBM traffic,
    removes the fp32r n<256 4x penalty, faster ldweights.
  - weights hoisted to SBUF once (v1 reloaded W_q/W_k 4x: 25MB extra DMA).
  - causal mask applied as a 0/1 fp16 multiply on the exp output (SBUF)
    instead of -1e30 add on the score PSUM: cheaper and shortens the
    PSUM critical path.
  - softmax denominator reciprocal via reciprocal_approx_fast (v1's
    nc.vector.reciprocal was 3.3us per call, 106us total DVE).
  - per-chunk emission interleave: projection chunk c+1 is emitted between
    attention chunk c and its output projection, so the tile scheduler can
    fill the (Activation-bound) attention phase with projection matmuls and
    keep the PE continuously busy at its top p-state.

Layout (per core):
  qpT/kpT: [dh-on-partitions (2 heads x 64), hp, S] fp16
  scoresT [sk, sq] in PSUM; exp'd (scale fused) to fp16 et; PV feeds from et
  directly; denominator = ones-column appended to V (row 64 of the PV psum);
  normalization = approx-reciprocal + gpsimd partition_broadcast + fp16 mul.
"""
import math
import numpy as np

import concourse.bass as bass
import concourse.tile as tile
import concourse.mybir as mybir
from concourse import bacc, bass_utils

# problem constants
B, S, D = 4, 2048, 1024
H, Dh = 16, 64
EQ, EV = 2048, 1024          # q/k and v input feature dims
F = 512                      # features per core (8 heads x 64)
P = 128
N_CORES = 8
SCALE = 1.0 / math.sqrt(D)   # 1/32
ROPE_BASE = 10000.0
SWAP_MASK = [i ^ 1 for i in range(32)]

F16 = mybir.dt.float16
F32 = mybir.dt.float32

# test hooks (harness ignores these)
KERNEL_TRACE = False
LAST_RESULT = None

_nc_cache = None


def _build_nc():
    nc = bacc.Bacc("TRN2", target_bir_lowering=False, debug=False)
    qT = nc.dram_tensor("qT", [EQ, S], F16, kind="ExternalInput").ap()
    kT = nc.dram_tensor("kT", [EQ, S], F16, kind="ExternalInput").ap()
    vT = nc.dram_tensor("vT", [EV, S], F16, kind="ExternalInput").ap()
    wqT = nc.dram_tensor("wqT", [EQ, F], F16, kind="ExternalInput").ap()
    wkT = nc.dram_tensor("wkT", [EQ, F], F16, kind="ExternalInput").ap()
    wvT = nc.dram_tensor("wvT", [EV, F], F16, kind="ExternalInput").ap()
    woT = nc.dram_tensor("woT", [F, D], F16, kind="ExternalInput").ap()
    cosf = nc.dram_tensor("cosf", [P, S], F16, kind="ExternalInput").ap()
    sinf = nc.dram_tensor("sinf", [P, S], F16, kind="ExternalInput").ap()
    maskA = nc.dram_tensor("maskA", [P, P], F16, kind="ExternalInput").ap()
    out = nc.dram_tensor("out", [S, D], F32, kind="ExternalOutput").ap()

    EXP = mybir.ActivationFunctionType.Exp

    with tile.TileContext(nc) as tc:
        with (
            tc.tile_pool(name="consts", bufs=1) as consts,
            tc.tile_pool(name="persist", bufs=1) as persist,
            tc.tile_pool(name="insb", bufs=2) as insb_pool,
            tc.tile_pool(name="vsb", bufs=2) as vsb_pool,
            tc.tile_pool(name="rope", bufs=2) as rope_pool,
            tc.tile_pool(name="et", bufs=3) as et_pool,
            tc.tile_pool(name="norm", bufs=2) as norm_pool,
            tc.tile_pool(name="denp", bufs=1) as den_pool,
            tc.tile_pool(name="attnc", bufs=2) as attnc_pool,
            tc.tile_pool(name="outsb", bufs=2) as out_pool,
            tc.tile_pool(name="projps", bufs=2, space="PSUM") as proj_ps,
            tc.tile_pool(name="scps", bufs=2, space="PSUM") as sc_ps,
            tc.tile_pool(name="pops", bufs=2, space="PSUM") as po_ps,
        ):
            # ---- persistent activations
            qpT = persist.tile([P, 4, S], F16, tag="qpT")
            kpT = persist.tile([P, 4, S], F16, tag="kpT")
            vpa = persist.tile([P, 16, 8, 65], F16, tag="vpa")
            nc.vector.memset(vpa[:, :, :, 64:65], 1.0)  # softmax-denominator ones

            # ---- weights + tables, loaded once
            wq_t = consts.tile([P, 16, F], F16, tag="wq")
            wk_t = consts.tile([P, 16, F], F16, tag="wk")
            wv_t = consts.tile([P, 8, F], F16, tag="wv")
            wo_t = consts.tile([P, 4, D], F16, tag="wo")
            cos_t = consts.tile([P, S], F16, tag="cos")
            sin_t = consts.tile([P, S], F16, tag="sin")
            mask_t = consts.tile([P, P], F16, tag="mask")
            # 3D views of the HBM inputs: [p, e, s]
            qTr = qT.rearrange("(e p) s -> p e s", p=P)
            kTr = kT.rearrange("(e p) s -> p e s", p=P)
            vTr = vT.rearrange("(e p) s -> p e s", p=P)
            wqTr = wqT.rearrange("(e p) f -> p e f", p=P)
            wkTr = wkT.rearrange("(e p) f -> p e f", p=P)
            wvTr = wvT.rearrange("(e p) f -> p e f", p=P)
            woTr = woT.rearrange("(e p) f -> p e f", p=P)

            def bulk(dmae, dst, src, n_e, split=4):
                step = n_e // split
                for i in range(0, n_e, step):
                    dmae.dma_start(dst[:, i:i + step], src[:, i:i + step])

            def load_qk(c, split=4):
                ssl = slice(c * 512, (c + 1) * 512)
                qsb = insb_pool.tile([P, 16, 512], F16, tag="insb", name=f"qsb{c}")
                bulk(nc.sync, qsb, qTr[:, :, ssl], 16, split)
                ksb = insb_pool.tile([P, 16, 512], F16, tag="insb", name=f"ksb{c}")
                bulk(nc.scalar, ksb, kTr[:, :, ssl], 16, split)
                return qsb, ksb

            def load_v(stq):
                vsb = vsb_pool.tile([P, 8, 512], F16, tag="vsb", name=f"vsb{stq}")
                bulk(nc.gpsimd, vsb, vTr[:, :, stq * 512:(stq + 1) * 512], 8, 2)
                return vsb

            # startup loads, ordered by chunk-0 criticality per queue:
            # gpsimd: wq -> wv -> vsb0 ; sync: qsb0 -> wo, mask ;
            # scalar: cos, sin -> ksb0 -> wk
            bulk(nc.gpsimd, wq_t, wqTr, 16, 8)
            nc.scalar.dma_start(cos_t[:], cosf)
            nc.scalar.dma_start(sin_t[:], sinf)
            qsb0, ksb0 = load_qk(0, split=8)
            bulk(nc.scalar, wk_t, wkTr, 16)
            bulk(nc.gpsimd, wv_t, wvTr, 8, 2)
            vsb0 = load_v(0)
            bulk(nc.sync, wo_t, woTr, 4, 2)
            nc.sync.dma_start(mask_t[:], maskA)

            def proj_qk(c, qsb, ksb):
                """Project q and k for s-chunk c (fp16 matmuls) + rope."""
                ssl = slice(c * 512, (c + 1) * 512)
                for xsb, w_t, dstT, nm in ((qsb, wq_t, qpT, "q"),
                                           (ksb, wk_t, kpT, "k")):
                    for ci in range(4):
                        ps = proj_ps.tile([P, 512], F32, tag="proj",
                                          name=f"ps{nm}{c}_{ci}")
                        for e in range(16):
                            nc.tensor.matmul(ps[:], w_t[:, e, ci * P:(ci + 1) * P],
                                             xsb[:, e, :],
                                             start=(e == 0), stop=(e == 15))
                        # rope: out = x*cos + pairswap(x*sin')
                        a16 = rope_pool.tile([P, 512], F16, tag="ropeA")
                        nc.scalar.copy(a16[:], ps[:])
                        cm = rope_pool.tile([P, 512], F16, tag="ropeC")
                        nc.vector.tensor_mul(cm[:], a16[:], cos_t[:, ssl])
                        sm = rope_pool.tile([P, 512], F16, tag="ropeS")
                        nc.vector.tensor_mul(sm[:], a16[:], sin_t[:, ssl])
                        sm2 = rope_pool.tile([P, 512], F16, tag="ropeS2")
                        nc.vector.stream_shuffle(sm2[:], sm[:], SWAP_MASK)
                        nc.vector.tensor_add(dstT[:, ci, ssl], cm[:], sm2[:])

            def proj_v(stq, vsb):
                """Project v for s-chunk stq into vpa (s on partitions)."""
                for j in range(4):
                    ps = proj_ps.tile([P, 512], F32, tag="proj",
                                      name=f"psv{stq}_{j}")
                    for e in range(8):
                        nc.tensor.matmul(ps[:], vsb[:, e, j * P:(j + 1) * P],
                                         wv_t[:, e, :],
                                         start=(e == 0), stop=(e == 7))
                    st = stq * 4 + j
                    nc.scalar.copy(vpa[:, st, :, 0:64],
                                   ps[:].rearrange("p (h d) -> p h d", h=8))

            def attn_hp(c, hp, attn_c):
                """Causal attention for query chunk c, head-pair hp."""
                nt = 4 * (c + 1)
                if True:
                    po_a = po_ps.tile([P, 512], F32, tag="po", name=f"poa{c}_{hp}")
                    po_b = po_ps.tile([P, 512], F32, tag="po", name=f"pob{c}_{hp}")
                    for t in range(nt):
                        tsl = slice(t * P, (t + 1) * P)
                        rr = P * (t - 4 * c) if t >= 4 * c else 0
                        qsl = slice(c * 512 + rr, (c + 1) * 512)
                        ps_s = sc_ps.tile([P, 2, 512], F32, tag="sc",
                                          name=f"scs{c}_{hp}_{t}")
                        nc.tensor.matmul(ps_s[:, 0, rr:512], kpT[0:64, hp, tsl],
                                         qpT[0:64, hp, qsl], start=True, stop=True)
                        nc.tensor.matmul(ps_s[:, 1, rr:512], kpT[64:128, hp, tsl],
                                         qpT[64:128, hp, qsl], start=True, stop=True)
                        et = et_pool.tile([P, 2, 512], F16, tag="et")
                        nc.scalar.activation(et[:, :, rr:512], ps_s[:, :, rr:512],
                                             EXP, scale=SCALE)
                        if t >= 4 * c:
                            # zero the above-diagonal triangle of this block
                            nc.vector.tensor_mul(
                                et[:, :, rr:rr + P], et[:, :, rr:rr + P],
                                mask_t[:, None, :].to_broadcast((P, 2, P)))
                        nc.tensor.matmul(po_a[0:65, rr:512], vpa[:, t, 2 * hp, :],
                                         et[:, 0, rr:512],
                                         start=(t == 0), stop=(t == nt - 1))
                        nc.tensor.matmul(po_b[0:65, rr:512], vpa[:, t, 2 * hp + 1, :],
                                         et[:, 1, rr:512],
                                         start=(t == 0), stop=(t == nt - 1))
                    # normalize: attn = po[0:64] * (1 / po[64]) per head.
                    # All DVE/gpsimd ops keep inputs at base partition 0
                    # (cross-base inputs miscompile; out-offset is fine).
                    denA = den_pool.tile([1, 512], F32, tag="denA")
                    nc.vector.tensor_copy(denA[:], po_a[64:65, :])
                    denB = den_pool.tile([1, 512], F32, tag="denB")
                    nc.vector.tensor_copy(denB[:], po_b[64:65, :])
                    rcfa = den_pool.tile([1, 512], F32, tag="rcfa")
                    nc.vector.reciprocal_approx_fast(out=rcfa[:], in_=denA[:])
                    rcfb = den_pool.tile([1, 512], F32, tag="rcfb")
                    nc.vector.reciprocal_approx_fast(out=rcfb[:], in_=denB[:])
                    # stage po down to fp16 SBUF immediately so the PSUM bank
                    # frees before the (longer) reciprocal/broadcast chain
                    posbA = norm_pool.tile([64, 512], F16, tag="posbA")
                    nc.vector.tensor_copy(posbA[:], po_a[0:64, :])
                    posbB = norm_pool.tile([64, 512], F16, tag="posbB")
                    nc.vector.tensor_copy(posbB[:], po_b[0:64, :])
                    bcA = norm_pool.tile([64, 512], F32, tag="bcA")
                    nc.gpsimd.partition_broadcast(bcA[:], rcfa[:])
                    bcB = norm_pool.tile([64, 512], F32, tag="bcB")
                    nc.gpsimd.partition_broadcast(bcB[:], rcfb[:])
                    nc.vector.tensor_mul(attn_c[0:64, hp, :], posbA[:], bcA[:])
                    nc.vector.tensor_mul(attn_c[64:128, hp, :], posbB[:], bcB[:])

            def outproj(c, attn_c):
                for j in range(4):
                    pw = [po_ps.tile([P, 512], F32, tag="po", name=f"pw{c}_{j}_{i}")
                          for i in range(2)]
                    for ci in range(4):
                        for oc in range(2):
                            nc.tensor.matmul(pw[oc][:],
                                             attn_c[:, ci, j * P:(j + 1) * P],
                                             wo_t[:, ci, oc * 512:(oc + 1) * 512],
                                             start=(ci == 0), stop=(ci == 3))
                    row = (4 * c + j) * P
                    for oc in range(2):
                        ot = out_pool.tile([P, 512], F32, tag="ot")
                        nc.vector.tensor_copy(ot[:], pw[oc][:])
                        nc.sync.dma_start(out[row:row + P, oc * 512:(oc + 1) * 512],
                                          ot[:])

            # ---- program: chunk 0 projections; per chunk: attention with
            # next-chunk projections emitted alongside (scheduler filler for
            # the PE); chunks 2 and 3 attention interleaved at head-pair
            # granularity so the combined window stays PE-bound (chunk 3
            # alone is exp/Activation-bound).
            proj_qk(0, qsb0, ksb0)
            proj_v(0, vsb0)
            for c in range(4):
                attn_c = attnc_pool.tile([P, 4, 512], F16, tag="attn",
                                         name=f"attn{c}")
                for hp in range(4):
                    attn_hp(c, hp, attn_c)
                if c < 3:
                    qsb, ksb = load_qk(c + 1)
                    vsb = load_v(c + 1)
                    proj_qk(c + 1, qsb, ksb)
                    proj_v(c + 1, vsb)
                outproj(c, attn_c)
    nc.compile()
    return nc


def _tables():
    inv = (1.0 / (ROPE_BASE ** (np.arange(0, Dh, 2, dtype=np.float32) / Dh))
           ).astype(np.float32)                      # [32]
    pos = np.arange(S, dtype=np.float32)
    ang = pos[:, None] * inv[None, :]                # [S, 32]
    cos = np.cos(ang).astype(np.float32)
    sin = np.sin(ang).astype(np.float32)
    d = np.arange(P) % Dh
    i = d // 2
    cosf = np.ascontiguousarray(cos[:, i].T).astype(np.float16)   # [128, S]
    sgn = np.where(d % 2 == 0, 1.0, -1.0).astype(np.float32)
    sinf = np.ascontiguousarray(sin[:, i].T * sgn[:, None]).astype(np.float16)

    p = np.arange(P)
    j = np.arange(P)
    maskA = np.where(p[:, None] <= j[None, :], 1.0, 0.0).astype(np.float16)
    return cosf, sinf, maskA


def kernel(q, k, v, W_q, W_k, W_v, W_o):
    global _nc_cache, LAST_RESULT
    if _nc_cache is None:
        _nc_cache = _build_nc()
    nc = _nc_cache

    cosf, sinf, maskA = _tables()
    q = np.asarray(q, dtype=np.float32)
    k = np.asarray(k, dtype=np.float32)
    v = np.asarray(v, dtype=np.float32)
    W_q = np.asarray(W_q, dtype=np.float32)
    W_k = np.asarray(W_k, dtype=np.float32)
    W_v = np.asarray(W_v, dtype=np.float32)
    W_o = np.asarray(W_o, dtype=np.float32)

    in_maps = []
    for b in range(B):
        qTb = np.ascontiguousarray(q[b].T).astype(np.float16)
        kTb = np.ascontiguousarray(k[b].T).astype(np.float16)
        vTb = np.ascontiguousarray(v[b].T).astype(np.float16)
        for g in range(2):
            fs = slice(g * F, (g + 1) * F)
            in_maps.append({
                "qT": qTb, "kT": kTb, "vT": vTb,
                "wqT": np.ascontiguousarray(W_q[fs, :].T).astype(np.float16),
                "wkT": np.ascontiguousarray(W_k[fs, :].T).astype(np.float16),
                "wvT": np.ascontiguousarray(W_v[fs, :].T).astype(np.float16),
                "woT": np.ascontiguousarray(W_o[:, fs].T).astype(np.float16),
                "cosf": cosf, "sinf": sinf, "maskA": maskA,
            })

    res = bass_utils.run_bass_kernel_spmd(
        nc, in_maps, core_ids=list(range(N_CORES)), trace=KERNEL_TRACE)
    LAST_RESULT = res

    final = np.empty((B, S, D), dtype=np.float32)
    for b in range(B):
        final[b] = res.results[2 * b]["out"] + res.results[2 * b + 1]["out"]
    return final


# revision 33
# speedup vs baseline: 1.0424x; 1.0034x over previous
"""TRN2 Bass kernel for nn_Attention_23493471109551 (v2, fp16).

Full attention layer: QKV projections + interleaved RoPE + causal softmax
attention + output projection, for B=4, S=2048, D=1024, H=16, Dh=64, fp32 I/O.

Sharding: 8 cores = 4 batches x 2 head-groups (8 heads each).  Each core
computes its batch/head-group's attention and a partial output projection
(W_o row-block); host sums the two partials per batch.

v2 changes vs v1 (fp32r baseline, 707us):
  - all matmul operands fp16 (err 3.4e-4 vs 2e-2 budget): halves HBM traffic,
    removes the fp32r n<256 4x penalty, faster ldweights.
  - weights hoisted to SBUF once (v1 reloaded W_q/W_k 4x: 25MB extra DMA).
  - causal mask applied as a 0/1 fp16 multiply on the exp output (SBUF)
    instead of -1e30 add on the score PSUM: cheaper and shortens the
    PSUM critical path.
  - softmax denominator reciprocal via reciprocal_approx_fast (v1's
    nc.vector.reciprocal was 3.3us per call, 106us total DVE).
  - per-chunk emission interleave: projection chunk c+1 is emitted between
    attention chunk c and its output projection, so the tile scheduler can
    fill the (Activation-bound) attention phase with projection matmuls and
    keep the PE continuously busy at its top p-state.

Layout (per core):
  qpT/kpT: [dh-on-partitions (2 heads x 64), hp, S] fp16
  scoresT [sk, sq] in PSUM; exp'd (scale fused) to fp16 et; PV feeds from et
  directly; denominator = ones-column appended to V (row 64 of the PV psum);
  normalization = approx-reciprocal + gpsimd partition_broadcast + fp16 mul.
"""
import math
import numpy as np

import concourse.bass as bass
import concourse.tile as tile
import concourse.mybir as mybir
from concourse import bacc, bass_utils

# problem constants
B, S, D = 4, 2048, 1024
H, Dh = 16, 64
EQ, EV = 2048, 1024          # q/k and v input feature dims
F = 512                      # features per core (8 heads x 64)
P = 128
N_CORES = 8
SCALE = 1.0 / math.sqrt(D)   # 1/32
ROPE_BASE = 10000.0
SWAP_MASK = [i ^ 1 for i in range(32)]

F16 = mybir.dt.float16
F32 = mybir.dt.float32

# test hooks (harness ignores these)
KERNEL_TRACE = False
LAST_RESULT = None

_nc_cache = None


def _build_nc():
    nc = bacc.Bacc("TRN2", target_bir_lowering=False, debug=False)
    qT = nc.dram_tensor("qT", [EQ, S], F16, kind="ExternalInput").ap()
    kT = nc.dram_tensor("kT", [EQ, S], F16, kind="ExternalInput").ap()
    vT = nc.dram_tensor("vT", [EV, S], F16, kind="ExternalInput").ap()
    wqT = nc.dram_tensor("wqT", [EQ, F], F16, kind="ExternalInput").ap()
    wkT = nc.dram_tensor("wkT", [EQ, F], F16, kind="ExternalInput").ap()
    wvT = nc.dram_tensor("wvT", [EV, F], F16, kind="ExternalInput").ap()
    woT = nc.dram_tensor("woT", [F, D], F16, kind="ExternalInput").ap()
    cosf = nc.dram_tensor("cosf", [P, S], F16, kind="ExternalInput").ap()
    sinf = nc.dram_tensor("sinf", [P, S], F16, kind="ExternalInput").ap()
    maskA = nc.dram_tensor("maskA", [P, P], F16, kind="ExternalInput").ap()
    out = nc.dram_tensor("out", [S, D], F32, kind="ExternalOutput").ap()

    EXP = mybir.ActivationFunctionType.Exp

    with tile.TileContext(nc) as tc:
        with (
            tc.tile_pool(name="consts", bufs=1) as consts,
            tc.tile_pool(name="persist", bufs=1) as persist,
            tc.tile_pool(name="insb", bufs=2) as insb_pool,
            tc.tile_pool(name="vsb", bufs=2) as vsb_pool,
            tc.tile_pool(name="rope", bufs=2) as rope_pool,
            tc.tile_pool(name="et", bufs=3) as et_pool,
            tc.tile_pool(name="norm", bufs=2) as norm_pool,
            tc.tile_pool(name="denp", bufs=1) as den_pool,
            tc.tile_pool(name="attnc", bufs=2) as attnc_pool,
            tc.tile_pool(name="outsb", bufs=2) as out_pool,
            tc.tile_pool(name="scps", bufs=2, space="PSUM") as sc_ps,
            tc.tile_pool(name="pops", bufs=2, space="PSUM") as po_ps,
        ):
            # released after the last projection so the tail (attention
            # chunk 3) can use its 2 banks for a deeper score pipeline
            proj_ps = tc.alloc_tile_pool(name="projps", bufs=2, space="PSUM")
            # ---- persistent activations
            qpT = persist.tile([P, 4, S], F16, tag="qpT")
            kpT = persist.tile([P, 4, S], F16, tag="kpT")
            vpa = persist.tile([P, 16, 8, 65], F16, tag="vpa")
            nc.vector.memset(vpa[:, :, :, 64:65], 1.0)  # softmax-denominator ones

            # ---- weights + tables, loaded once
            wq_t = consts.tile([P, 16, F], F16, tag="wq")
            wk_t = consts.tile([P, 16, F], F16, tag="wk")
            wv_t = consts.tile([P, 8, F], F16, tag="wv")
            wo_t = consts.tile([P, 4, D], F16, tag="wo")
            cos_t = consts.tile([P, S], F16, tag="cos")
            sin_t = consts.tile([P, S], F16, tag="sin")
            mask_t = consts.tile([P, P], F16, tag="mask")
            # 3D views of the HBM inputs: [p, e, s]
            qTr = qT.rearrange("(e p) s -> p e s", p=P)
            kTr = kT.rearrange("(e p) s -> p e s", p=P)
            vTr = vT.rearrange("(e p) s -> p e s", p=P)
            wqTr = wqT.rearrange("(e p) f -> p e f", p=P)
            wkTr = wkT.rearrange("(e p) f -> p e f", p=P)
            wvTr = wvT.rearrange("(e p) f -> p e f", p=P)
            woTr = woT.rearrange("(e p) f -> p e f", p=P)

            def bulk(dmae, dst, src, n_e, split=4):
                step = n_e // split
                for i in range(0, n_e, step):
                    dmae.dma_start(dst[:, i:i + step], src[:, i:i + step])

            def load_qk(c, split=4):
                ssl = slice(c * 512, (c + 1) * 512)
                qsb = insb_pool.tile([P, 16, 512], F16, tag="insb", name=f"qsb{c}")
                bulk(nc.sync, qsb, qTr[:, :, ssl], 16, split)
                ksb = insb_pool.tile([P, 16, 512], F16, tag="insb", name=f"ksb{c}")
                bulk(nc.scalar, ksb, kTr[:, :, ssl], 16, split)
                return qsb, ksb

            def load_v(stq):
                vsb = vsb_pool.tile([P, 8, 512], F16, tag="vsb", name=f"vsb{stq}")
                bulk(nc.gpsimd, vsb, vTr[:, :, stq * 512:(stq + 1) * 512], 8, 2)
                return vsb

            # startup loads, ordered by chunk-0 criticality per queue:
            # gpsimd: wq -> wv -> vsb0 ; sync: qsb0 -> wo, mask ;
            # scalar: cos, sin -> ksb0 -> wk
            bulk(nc.gpsimd, wq_t, wqTr, 16, 8)
            nc.scalar.dma_start(cos_t[:], cosf)
            nc.scalar.dma_start(sin_t[:], sinf)
            ssl0 = slice(0, 512)
            qsb0 = insb_pool.tile([P, 16, 512], F16, tag="insb", name="qsb0")
            bulk(nc.sync, qsb0[:, 0:8], qTr[:, 0:8, ssl0], 8, 4)
            bulk(nc.scalar, qsb0[:, 8:16], qTr[:, 8:16, ssl0], 8, 4)
            ksb0 = insb_pool.tile([P, 16, 512], F16, tag="insb", name="ksb0")
            bulk(nc.scalar, ksb0, kTr[:, :, ssl0], 16, 4)
            bulk(nc.scalar, wk_t, wkTr, 16)
            bulk(nc.gpsimd, wv_t, wvTr, 8, 2)
            vsb0 = load_v(0)
            bulk(nc.sync, wo_t, woTr, 4, 2)
            nc.sync.dma_start(mask_t[:], maskA)

            def proj_qk(c, qsb, ksb):
                """Project q and k for s-chunk c (fp16 matmuls) + rope."""
                ssl = slice(c * 512, (c + 1) * 512)
                for xsb, w_t, dstT, nm in ((qsb, wq_t, qpT, "q"),
                                           (ksb, wk_t, kpT, "k")):
                    for ci in range(4):
                        ps = proj_ps.tile([P, 512], F32, tag="proj",
                                          name=f"ps{nm}{c}_{ci}")
                        for e in range(16):
                            nc.tensor.matmul(ps[:], w_t[:, e, ci * P:(ci + 1) * P],
                                             xsb[:, e, :],
                                             start=(e == 0), stop=(e == 15))
                        # rope: out = x*cos + pairswap(x*sin')
                        a16 = rope_pool.tile([P, 512], F16, tag="ropeA")
                        nc.scalar.copy(a16[:], ps[:])
                        cm = rope_pool.tile([P, 512], F16, tag="ropeC")
                        nc.vector.tensor_mul(cm[:], a16[:], cos_t[:, ssl])
                        sm = rope_pool.tile([P, 512], F16, tag="ropeS")
                        nc.vector.tensor_mul(sm[:], a16[:], sin_t[:, ssl])
                        sm2 = rope_pool.tile([P, 512], F16, tag="ropeS2")
                        nc.vector.stream_shuffle(sm2[:], sm[:], SWAP_MASK)
                        nc.vector.tensor_add(dstT[:, ci, ssl], cm[:], sm2[:])

            def proj_v(stq, vsb):
                """Project v for s-chunk stq into vpa (s on partitions)."""
                for j in range(4):
                    ps = proj_ps.tile([P, 512], F32, tag="proj",
                                      name=f"psv{stq}_{j}")
                    for e in range(8):
                        nc.tensor.matmul(ps[:], vsb[:, e, j * P:(j + 1) * P],
                                         wv_t[:, e, :],
                                         start=(e == 0), stop=(e == 7))
                    st = stq * 4 + j
                    nc.scalar.copy(vpa[:, st, :, 0:64],
                                   ps[:].rearrange("p (h d) -> p h d", h=8))

            def attn_hp(c, hp, attn_c, sc_pools=None):
                """Causal attention for query chunk c, head-pair hp."""
                nt = 4 * (c + 1)
                sc_pools = sc_pools or [sc_ps]
                if True:
                    po_a = po_ps.tile([P, 512], F32, tag="po", name=f"poa{c}_{hp}")
                    po_b = po_ps.tile([P, 512], F32, tag="po", name=f"pob{c}_{hp}")
                    for t in range(nt):
                        tsl = slice(t * P, (t + 1) * P)
                        rr = P * (t - 4 * c) if t >= 4 * c else 0
                        qsl = slice(c * 512 + rr, (c + 1) * 512)
                        ps_s = sc_pools[t % len(sc_pools)].tile(
                            [P, 2, 512], F32, tag="sc", name=f"scs{c}_{hp}_{t}")
                        nc.tensor.matmul(ps_s[:, 0, rr:512], kpT[0:64, hp, tsl],
                                         qpT[0:64, hp, qsl], start=True, stop=True)
                        nc.tensor.matmul(ps_s[:, 1, rr:512], kpT[64:128, hp, tsl],
                                         qpT[64:128, hp, qsl], start=True, stop=True)
                        et = et_pool.tile([P, 2, 512], F16, tag="et")
                        nc.scalar.activation(et[:, :, rr:512], ps_s[:, :, rr:512],
                                             EXP, scale=SCALE)
                        if t >= 4 * c:
                            # zero the above-diagonal triangle of this block
                            nc.vector.tensor_mul(
                                et[:, :, rr:rr + P], et[:, :, rr:rr + P],
                                mask_t[:, None, :].to_broadcast((P, 2, P)))
                        nc.tensor.matmul(po_a[0:65, rr:512], vpa[:, t, 2 * hp, :],
                                         et[:, 0, rr:512],
                                         start=(t == 0), stop=(t == nt - 1))
                        nc.tensor.matmul(po_b[0:65, rr:512], vpa[:, t, 2 * hp + 1, :],
                                         et[:, 1, rr:512],
                                         start=(t == 0), stop=(t == nt - 1))
                    # normalize: attn = po[0:64] * (1 / po[64]) per head.
                    # All DVE/gpsimd ops keep inputs at base partition 0
                    # (cross-base inputs miscompile; out-offset is fine).
                    denA = den_pool.tile([1, 512], F32, tag="denA")
                    nc.vector.tensor_copy(denA[:], po_a[64:65, :])
                    denB = den_pool.tile([1, 512], F32, tag="denB")
                    nc.vector.tensor_copy(denB[:], po_b[64:65, :])
                    rcfa = den_pool.tile([1, 512], F32, tag="rcfa")
                    nc.vector.reciprocal_approx_fast(out=rcfa[:], in_=denA[:])
                    rcfb = den_pool.tile([1, 512], F32, tag="rcfb")
                    nc.vector.reciprocal_approx_fast(out=rcfb[:], in_=denB[:])
                    # stage po down to fp16 SBUF immediately so the PSUM bank
                    # frees before the (longer) reciprocal/broadcast chain
                    posbA = norm_pool.tile([64, 512], F16, tag="posbA")
                    nc.vector.tensor_copy(posbA[:], po_a[0:64, :])
                    posbB = norm_pool.tile([64, 512], F16, tag="posbB")
                    nc.vector.tensor_copy(posbB[:], po_b[0:64, :])
                    bcA = norm_pool.tile([64, 512], F32, tag="bcA")
                    nc.gpsimd.partition_broadcast(bcA[:], rcfa[:])
                    bcB = norm_pool.tile([64, 512], F32, tag="bcB")
                    nc.gpsimd.partition_broadcast(bcB[:], rcfb[:])
                    nc.vector.tensor_mul(attn_c[0:64, hp, :], posbA[:], bcA[:])
                    nc.vector.tensor_mul(attn_c[64:128, hp, :], posbB[:], bcB[:])

            def outproj(c, attn_c):
                for j in range(4):
                    pw = [po_ps.tile([P, 512], F32, tag="po", name=f"pw{c}_{j}_{i}")
                          for i in range(2)]
                    for ci in range(4):
                        for oc in range(2):
                            nc.tensor.matmul(pw[oc][:],
                                             attn_c[:, ci, j * P:(j + 1) * P],
                                             wo_t[:, ci, oc * 512:(oc + 1) * 512],
                                             start=(ci == 0), stop=(ci == 3))
                    row = (4 * c + j) * P
                    for oc in range(2):
                        ot = out_pool.tile([P, 512], F32, tag="ot")
                        nc.vector.tensor_copy(ot[:], pw[oc][:])
                        nc.sync.dma_start(out[row:row + P, oc * 512:(oc + 1) * 512],
                                          ot[:])

            # ---- program: chunk 0 projections; per chunk: attention with
            # next-chunk projections emitted alongside (scheduler filler for
            # the PE); chunks 2 and 3 attention interleaved at head-pair
            # granularity so the combined window stays PE-bound (chunk 3
            # alone is exp/Activation-bound).
            proj_qk(0, qsb0, ksb0)
            proj_v(0, vsb0)
            for c in range(3):
                attn_c = attnc_pool.tile([P, 4, 512], F16, tag="attn",
                                         name=f"attn{c}")
                for hp in range(4):
                    attn_hp(c, hp, attn_c)
                qsb, ksb = load_qk(c + 1)
                vsb = load_v(c + 1)
                proj_qk(c + 1, qsb, ksb)
                proj_v(c + 1, vsb)
                if c == 2:
                    proj_ps.release()
                outproj(c, attn_c)
            # tail: attention chunk 3 with a 3-deep score rotation using the
            # banks released by the projection pool
            tail_ps = tc.alloc_tile_pool(name="tailps", bufs=1, space="PSUM")
            attn_c = attnc_pool.tile([P, 4, 512], F16, tag="attn", name="attn3")
            for hp in range(4):
                attn_hp(3, hp, attn_c, sc_pools=[sc_ps, sc_ps, tail_ps])
            outproj(3, attn_c)
            tail_ps.release()
    nc.compile()
    return nc


def _tables():
    inv = (1.0 / (ROPE_BASE ** (np.arange(0, Dh, 2, dtype=np.float32) / Dh))
           ).astype(np.float32)                      # [32]
    pos = np.arange(S, dtype=np.float32)
    ang = pos[:, None] * inv[None, :]                # [S, 32]
    cos = np.cos(ang).astype(np.float32)
    sin = np.sin(ang).astype(np.float32)
    d = np.arange(P) % Dh
    i = d // 2
    cosf = np.ascontiguousarray(cos[:, i].T).astype(np.float16)   # [128, S]
    sgn = np.where(d % 2 == 0, 1.0, -1.0).astype(np.float32)
    sinf = np.ascontiguousarray(sin[:, i].T * sgn[:, None]).astype(np.float16)

    p = np.arange(P)
    j = np.arange(P)
    maskA = np.where(p[:, None] <= j[None, :], 1.0, 0.0).astype(np.float16)
    return cosf, sinf, maskA


def kernel(q, k, v, W_q, W_k, W_v, W_o):
    global _nc_cache, LAST_RESULT
    if _nc_cache is None:
        _nc_cache = _build_nc()
    nc = _nc_cache

    cosf, sinf, maskA = _tables()
    q = np.asarray(q, dtype=np.float32)
    k = np.asarray(k, dtype=np.float32)
    v = np.asarray(v, dtype=np.float32)
    W_q = np.asarray(W_q, dtype=np.float32)
    W_k = np.asarray(W_k, dtype=np.float32)
    W_v = np.asarray(W_v, dtype=np.float32)
    W_o = np.asarray(W_o, dtype=np.float32)

    in_maps = []
    for b in range(B):
        qTb = np.ascontiguousarray(q[b].T).astype(np.float16)
        kTb = np.ascontiguousarray(k[b].T).astype(np.float16)
        vTb = np.ascontiguousarray(v[b].T).astype(np.float16)
        for g in range(2):
            fs = slice(g * F, (g + 1) * F)
            in_maps.append({
                "qT": qTb, "kT": kTb, "vT": vTb,
                "wqT": np.ascontiguousarray(W_q[fs, :].T).astype(np.float16),
                "wkT": np.ascontiguousarray(W_k[fs, :].T).astype(np.float16),
                "wvT": np.ascontiguousarray(W_v[fs, :].T).astype(np.float16),
                "woT": np.ascontiguousarray(W_o[:, fs].T).astype(np.float16),
                "cosf": cosf, "sinf": sinf, "maskA": maskA,
            })

    res = bass_utils.run_bass_kernel_spmd(
        nc, in_maps, core_ids=list(range(N_CORES)), trace=KERNEL_TRACE)
    LAST_RESULT = res

    final = np.empty((B, S, D), dtype=np.float32)
    for b in range(B):
        final[b] = res.results[2 * b]["out"] + res.results[2 * b + 1]["out"]
    return final
